# revision 4
# baseline (speedup 1.0000x reference)
"""EnvLSTM Trainium2 kernel v2 (nn_EnvLSTM_86577950753068).

Bidirectional gated scan, time-parallel Picard iteration, all-fp16 refresh
form:
  - 8 cores: 0-3 left scan (time blocks 0-3), 4-7 right scan (reversed input,
    block-paired so core 4+j holds original-time block j).
  - Sweep s: A = WX + U @ C_{s-1} recomputed from scratch each sweep (fp16
    operands, f32 PSUM accumulate).  With fp16-stored C this matches the
    residual form numerically (telescoping) but needs no A storage, no DRAM
    round trips, and no delta computation: the matmul rhs is a direct SBUF
    slice of the previous C buffer.
  - Gate biases are applied via the activation bias operand (per-partition),
    killing the bias matmuls.
  - Per row-block jo the scan is emitted inline right after jo's gates, so
    scans overlap the matmul phase and the inter-sweep PE stall is one
    act+mul+scan (~2us).
  - Boundary states travel via a per-sweep MEM-sized fp16 AllGather
    (one-sweep delay); output projection tanh(wo@lenv + uo@renv + bo) after a
    pairwise fp16 env AllGather, split by columns (left core j computes
    out[:, 0:512], right core j+4 computes out[:, 512:1024] of block j).
  - Schedule: p1 + 12 r sweeps -> rel err ~7e-3 in emulation (gate 2e-2).
"""

import os
import numpy as np
import ml_dtypes

import concourse.bass as bass
import concourse.mybir as mybir
import concourse.tile as tile
from concourse.bass_utils import run_bass_kernel_spmd

F32 = mybir.dt.float32
F16 = mybir.dt.float16
F8 = mybir.dt.float8e4
AF = mybir.ActivationFunctionType
ALU = mybir.AluOpType
DR = mybir.MatmulPerfMode.DoubleRow

T, IN, MEM, OUT = 4096, 1024, 1024, 1024
TC = 1024              # time steps per core
P = 128                # partitions
NK = 8                 # contraction k-tiles (MEM/128)
NGT = 24               # gate tiles (3*MEM/128)
NCORES = 8
# p1 + 7 fp8 DoubleRow sweeps + 5 fp16 sweeps; emulated rel err 1.16e-2,
# measured HW err has run 1.5-1.8x better than emulated on fp8 schedules
SCHEDULE = ['p1'] + ['r8'] * 7 + ['r'] * 5
NSWEEP = len(SCHEDULE)


def legalize_waits(nc, max_w=1, max_u=1, max_w_engine=1):
    """This toolchain's walrus accepts at most one sync wait/update per
    instruction; split extras onto same-engine NoOp carriers (engine program
    order preserves semantics)."""
    n_nops = 0
    for bb in nc.main_func.blocks:
        out = []
        for ins in bb.instructions:
            si = ins.sync_info
            if si is None:
                out.append(ins)
                continue
            waits = list(si.on_wait or [])
            ups = list(si.on_update or [])
            cap = max_w if isinstance(ins, mybir.InstDMACopy) else max_w_engine
            pre, post = [], []
            if len(waits) > cap:
                for w in waits[:-cap]:
                    nop = mybir.InstNoOp(name=f"{ins.name}_lw{n_nops}", ins=[], outs=[])
                    n_nops += 1
                    nop.engine = ins.engine
                    nop.sync_info = mybir.SyncInfo(on_wait=[w], on_update=[])
                    pre.append(nop)
                waits = waits[-cap:]
            if len(ups) > max_u:
                for u in ups[max_u:]:
                    nop = mybir.InstNoOp(name=f"{ins.name}_lu{n_nops}", ins=[], outs=[])
                    n_nops += 1
                    nop.engine = ins.engine
                    nop.sync_info = mybir.SyncInfo(on_wait=[], on_update=[u])
                    post.append(nop)
                ups = ups[:max_u]
            ins.sync_info = mybir.SyncInfo(on_wait=waits, on_update=ups)
            out.extend(pre)
            out.append(ins)
            out.extend(post)
        bb.instructions = out
    return n_nops


def build_kernel(schedule=SCHEDULE):
    nc = bass.Bass("TRN2", target_bir_lowering=False, debug=False,
                   num_devices=1 if os.environ.get("KERNEL_SIM_NOCC", "0") == "1"
                   else NCORES)

    dt_x = nc.dram_tensor("xT16", [IN, TC], F16, kind="ExternalInput").ap()
    dt_w = nc.dram_tensor("w16", [IN, 3 * MEM], F16, kind="ExternalInput").ap()
    dt_u = nc.dram_tensor("u16", [MEM, 3 * MEM], F16, kind="ExternalInput").ap()
    dt_u8 = nc.dram_tensor("u8", [MEM, 3 * MEM], F8, kind="ExternalInput").ap()
    # per-core half of the output weights (left cores cols 0:512, right 512:)
    dt_wouo = nc.dram_tensor("wouo16", [2, MEM, 512], F16,
                             kind="ExternalInput").ap()
    # cols 0:24 per-gate-tile bias columns, 24:536 bo half, 536:664 ones
    dt_consts = nc.dram_tensor("consts", [P, 664], F32, kind="ExternalInput").ap()
    dt_sel = nc.dram_tensor("sel", [P, 8], F32, kind="ExternalInput").ap()
    dt_c0v = nc.dram_tensor("c0_vec", [P, 8], F32, kind="ExternalInput").ap()
    dt_c0h = nc.dram_tensor("c0_head", [P, 8], F32, kind="ExternalInput").ap()
    dt_out = nc.dram_tensor("out_part", [TC, 512], F32, kind="ExternalOutput").ap()

    with tile.TileContext(nc) as tc:
        with tc.tile_pool(name="persist", bufs=1) as persist:
            u_sb = persist.tile([P, NK, 3 * MEM], F16, name="u_sb")
            u8_sb = persist.tile([P, NK, 3 * MEM], F8, name="u8_sb")
            c8 = persist.tile([P, NK, TC], F8, name="c8")
            wx_sb = persist.tile([P, NGT, TC], F16, name="wx_sb")
            c_a = persist.tile([P, NK, TC + 1], F16, name="c_a")
            c_b = persist.tile([P, NK, TC + 1], F16, name="c_b")
            u_buf = persist.tile([P, NK, TC + 1], F16, name="u_buf")
            ci_a = persist.tile([P, NK], F32, name="ci_a")
            ci_b = persist.tile([P, NK], F32, name="ci_b")
            sel_sb = persist.tile([P, 8], F32, name="sel_sb")
            c0v_sb = persist.tile([P, NK], F32, name="c0v_sb")
            c0h_sb = persist.tile([P, NK], F32, name="c0h_sb")
            bnd_all = persist.tile([P, NK, 8], F16, name="bnd_all")
            bnd_tmp = persist.tile([P, NK, 8], F32, name="bnd_tmp")
            const_sb = persist.tile([P, 664], F32, name="const_sb")
            _body(nc, tc, locals())
    return nc


def _body(nc, tc, env):
    u_sb = env["u_sb"]; u8_sb = env["u8_sb"]; c8 = env["c8"]
    wx_sb = env["wx_sb"]; c_a = env["c_a"]; c_b = env["c_b"]
    u_buf = env["u_buf"]; ci_a = env["ci_a"]; ci_b = env["ci_b"]
    sel_sb = env["sel_sb"]; c0v_sb = env["c0v_sb"]; c0h_sb = env["c0h_sb"]
    bnd_all = env["bnd_all"]; bnd_tmp = env["bnd_tmp"]; const_sb = env["const_sb"]
    schedule = env["schedule"]
    dt_x = env["dt_x"]; dt_w = env["dt_w"]; dt_u = env["dt_u"]
    dt_u8 = env["dt_u8"]
    dt_wouo = env["dt_wouo"]; dt_consts = env["dt_consts"]
    dt_sel = env["dt_sel"]; dt_c0v = env["dt_c0v"]; dt_c0h = env["dt_c0h"]
    dt_out = env["dt_out"]
    S = len(schedule)

    with tc.tile_pool(name="dram", bufs=1, space="DRAM") as dram:
        bnd_in = dram.tile([1, MEM], F16, name="bnd_in")
        bnd_out = dram.tile([NCORES, MEM], F16, name="bnd_out")
        env_in = dram.tile([MEM, TC], F16, name="env_in")
        env_out = dram.tile([2, MEM, TC], F16, name="env_out")

        with (
            tc.tile_pool(name="psum", bufs=8, space="PSUM") as psum_pool,
            tc.tile_pool(name="stg_w", bufs=16) as stg_w,    # p1 w lhsT [P,128] f16
            tc.tile_pool(name="stg16", bufs=12) as stg16,    # rotating f16 [P,512]
            tc.tile_pool(name="stg32", bufs=4) as stg32,     # f32 [P,512] out tiles
        ):
            # ---- load persistent inputs ----
            # small/urgent + x on sync queue; bulk U on pool queue (overlaps p1)
            nc.sync.dma_start(const_sb[:, :], dt_consts)
            nc.sync.dma_start(sel_sb[:, :], dt_sel)
            nc.sync.dma_start(c0v_sb[:, :], dt_c0v)
            nc.sync.dma_start(c0h_sb[:, :], dt_c0h)
            # p1 reads x from c_b (the C buffer p1 neither reads nor writes);
            # sweep 1's activations overwrite it afterwards (WAR-ordered).
            nc.sync.dma_start(c_b[:, :, 0:TC],
                              dt_x.rearrange("(k p) t -> p k t", p=P))
            nc.gpsimd.dma_start(
                u8_sb[:, :, :], dt_u8.rearrange("(k p) g -> p k g", p=P))
            nc.gpsimd.dma_start(
                u_sb[:, :, :], dt_u.rearrange("(k p) g -> p k g", p=P))

            nc.gpsimd.memset(u_buf[:, :, 0], 0.0)
            nc.vector.tensor_copy(ci_a[:, :], c0v_sb[:, :])

            cbufs = [c_a, c_b]
            cins = [ci_a, ci_b]
            skip_mm = os.environ.get("KERNEL_SKIP_MM", "0") == "1"
            skip_scan = os.environ.get("KERNEL_SKIP_SCAN", "0") == "1"
            skip_bnd = os.environ.get("KERNEL_SKIP_BND", "0") == "1"
            no_ag = os.environ.get("KERNEL_NO_AG", "0") == "1"

            for s in range(S):
                mode = schedule[s]
                nxt_fp8 = s + 1 < S and schedule[s + 1] == 'r8'
                c_next = cbufs[s % 2]       # written by this sweep's scans
                c_cur = cbufs[(s + 1) % 2]  # C_{s-1} (x for p1)
                ci = cins[s % 2]
                ci_nxt = cins[(s + 1) % 2]

                nc.gpsimd.memset(c_next[:, :, 0], 1.0)  # identity scan coeff

                for jo in range(NK) if not skip_mm else []:
                    i_st = {}
                    for gate in range(3):
                        gt = gate * NK + jo
                        g0 = gt * P
                        ps = [psum_pool.tile([P, 512], F32,
                                             name=f"ps{s}_{gt}_{tch}", tag="ps")
                              for tch in range(2)]
                        if mode == 'r8':
                            # fp8 DoubleRow: 2 k-tiles per MM, 2 fp8/PE cell
                            for k in range(0, NK, 2):
                                lhsT = u8_sb[:, k:k + 2, g0:g0 + P]
                                for tch in range(2):
                                    t0 = tch * 512
                                    nc.tensor.matmul(
                                        ps[tch][:, :], lhsT,
                                        c8[:, k:k + 2, t0:t0 + 512],
                                        start=(k == 0), stop=(k == NK - 2),
                                        perf_mode=DR)
                        else:
                            for k in range(NK):
                                if mode == 'p1':
                                    wt = stg_w.tile([P, P], F16,
                                                    name=f"w{gt}_{k}", tag="w")
                                    nc.sync.dma_start(
                                        wt[:, :],
                                        dt_w[k * P:(k + 1) * P, g0:g0 + P])
                                    lhsT = wt[:, :]
                                else:
                                    lhsT = u_sb[:, k, g0:g0 + P]
                                for tch in range(2):
                                    t0 = tch * 512
                                    rhs = c_cur[:, k, t0:t0 + 512]
                                    nc.tensor.matmul(
                                        ps[tch][:, :], lhsT, rhs,
                                        start=(k == 0), stop=(k == NK - 1))
                        bias = const_sb[:, gt:gt + 1]
                        for tch in range(2):
                            t0 = tch * 512
                            if mode == 'p1':
                                nc.vector.tensor_copy(
                                    wx_sb[:, gt, t0:t0 + 512], ps[tch][:, :])
                                src = wx_sb[:, gt, t0:t0 + 512]
                            else:
                                a_st = stg16.tile([P, 512], F16,
                                                  name=f"a{s}_{gt}_{tch}",
                                                  tag="s16")
                                nc.vector.tensor_add(
                                    a_st[:, :], ps[tch][:, :],
                                    wx_sb[:, gt, t0:t0 + 512])
                                src = a_st[:, :]
                            if gate == 0:    # forget gate -> scan coeff
                                nc.scalar.activation(
                                    c_next[:, jo, 1 + t0:1 + t0 + 512],
                                    src, AF.Sigmoid, bias=bias)
                            elif gate == 1:  # input gate
                                sti = stg16.tile([P, 512], F16,
                                                 name=f"i{s}_{jo}_{tch}",
                                                 tag="s16")
                                nc.scalar.activation(sti[:, :], src,
                                                     AF.Sigmoid, bias=bias)
                                i_st[tch] = sti
                            else:            # candidate -> u = i*g
                                stg = stg16.tile([P, 512], F16,
                                                 name=f"g{s}_{jo}_{tch}",
                                                 tag="s16")
                                nc.scalar.activation(stg[:, :], src,
                                                     AF.Tanh, bias=bias)
                                nc.vector.tensor_mul(
                                    u_buf[:, jo, 1 + t0:1 + t0 + 512],
                                    i_st[tch][:, :], stg[:, :])
                    # ---- scan for this row block, inline ----
                    if not skip_scan:
                        nc.vector.tensor_tensor_scan(
                            c_next[:, jo, :],
                            c_next[:, jo, :],
                            u_buf[:, jo, :],
                            ci[:, jo:jo + 1],
                            ALU.mult, ALU.add)
                    if nxt_fp8:
                        # fp8 copy of C_s for the next sweep's DoubleRow MMs
                        nc.vector.tensor_copy(c8[:, jo, :],
                                              c_next[:, jo, 0:TC])

                # ---- boundary exchange (skip on last sweep) ----
                if s < S - 1 and skip_bnd:
                    nc.vector.tensor_copy(ci_nxt[:, :], c0v_sb[:, :])
                if s < S - 1 and not skip_bnd:
                    nc.sync.dma_start(
                        bnd_in.rearrange("o (j p) -> p (o j)", p=P),
                        c_next[:, :, TC])
                    if os.environ.get("KERNEL_SIM_NOCC", "0") != "1" and not no_ag:
                        nc.gpsimd.collective_compute(
                            "AllGather", ALU.bypass,
                            replica_groups=[list(range(NCORES))],
                            ins=[bnd_in.opt()],
                            outs=[bnd_out.opt()])
                    for r in range(NCORES):
                        nc.sync.dma_start(
                            bnd_all[:, :, r],
                            bnd_out[r:r + 1, :].rearrange(
                                "o (j p) -> p (o j)", p=P))
                    nc.vector.tensor_mul(
                        bnd_tmp[:, :, :], bnd_all[:, :, :],
                        sel_sb[:, :].unsqueeze(1).broadcast_to([P, NK, 8]))
                    nc.vector.tensor_reduce(
                        ci_nxt[:, :].unsqueeze(2), bnd_tmp[:, :, :],
                        op=ALU.add, axis=mybir.AxisListType.X)
                    nc.vector.tensor_add(ci_nxt[:, :], ci_nxt[:, :],
                                         c0h_sb[:, :])

            # ---- output stage ----
            if os.environ.get("KERNEL_SIM_NOOUT", "0") == "1":
                return
            c_fin = cbufs[(S - 1) % 2]
            nc.sync.dma_start(
                env_in.rearrange("(j p) t -> p j t", p=P),
                c_fin[:, :, 0:TC])
            if os.environ.get("KERNEL_SIM_NOCC", "0") != "1":
                nc.gpsimd.collective_compute(
                    "AllGather", ALU.bypass,
                    replica_groups=[[0, 4], [1, 5], [2, 6], [3, 7]],
                    ins=[env_in.opt()],
                    outs=[env_out.opt()])

            # each core computes its half of the output columns; host pairs
            # left core j (cols 0:512) with right core j+4 (cols 512:1024).
            pso = [psum_pool.tile([P, 512], F32, name=f"po_{i}", tag="ps")
                   for i in range(8)]
            for slot in (0, 1):
                for mk in range(NK):
                    wt = stg16.tile([P, 512], F16, name=f"ow_{slot}_{mk}",
                                    tag="s16")
                    nc.sync.dma_start(
                        wt[:, :], dt_wouo[slot, mk * P:(mk + 1) * P, :])
                    for half in range(2):
                        # slot 1 holds the right scan in reversed-time (local)
                        # order: original half h is local half (1-h), col-rev.
                        lhalf = half if slot == 0 else 1 - half
                        es = stg16.tile([P, 512], F16,
                                        name=f"oe_{slot}_{mk}_{half}",
                                        tag="s16")
                        nc.sync.dma_start(
                            es[:, :],
                            env_out[slot, mk * P:(mk + 1) * P,
                                    lhalf * 512:(lhalf + 1) * 512])
                        if slot == 0:
                            esu = es
                        else:
                            esu = stg16.tile([P, 512], F16,
                                             name=f"oer_{slot}_{mk}_{half}",
                                             tag="s16")
                            nc.vector.tensor_copy(esu[:, :], es[:, ::-1])
                        for q in range(4):
                            i = half * 4 + q
                            nc.tensor.matmul(pso[i][:, :],
                                             esu[:, q * P:(q + 1) * P],
                                             wt[:, :],
                                             start=(slot == 0 and mk == 0),
                                             stop=False)
            for i in range(8):
                # + bo via rank-1 ones @ bo matmul (bias along free dim)
                nc.tensor.matmul(pso[i][:, :], const_sb[0:1, 536:664],
                                 const_sb[0:1, 24:536],
                                 start=False, stop=True)
                ot = stg32.tile([P, 512], F32, name=f"oo_{i}", tag="s32")
                nc.scalar.activation(ot[:, :], pso[i][:, :], AF.Tanh)
                nc.sync.dma_start(dt_out[i * P:(i + 1) * P, :], ot[:, :])


def _prep_inputs(inputs):
    """Build the 8 per-core input maps from the full problem inputs."""
    f16 = np.float16
    x = np.ascontiguousarray(inputs["x"], dtype=np.float32)
    maps = []

    wouoT_full = np.stack([np.ascontiguousarray(inputs["wo"].T),
                           np.ascontiguousarray(inputs["uo"].T)]).astype(np.float32)
    bo = np.ascontiguousarray(inputs["bo"], np.float32).reshape(OUT)

    for c in range(NCORES):
        side = "l" if c < 4 else "r"
        w_all = np.concatenate([inputs[f"w{g}_{side}"] for g in ("f", "i", "c")], 0)
        u_all = np.concatenate([inputs[f"u{g}_{side}"] for g in ("f", "i", "c")], 0)
        b_all = np.concatenate([inputs[f"b{g}_{side}"] for g in ("f", "i", "c")], 0)
        c0 = np.asarray(inputs[f"c0_{side}"], np.float32)

        if c < 4:
            blk = c
            x_loc = x[TC * blk: TC * (blk + 1)]
            prev = c - 1 if c > 0 else None
            head = c == 0
        else:
            j = c - 4
            x_loc = x[TC * j: TC * (j + 1)][::-1]
            prev = c + 1 if j < 3 else None
            head = c == 7

        sel = np.zeros((P, 8), np.float32)
        if prev is not None:
            sel[:, prev] = 1.0
        c0_vec = np.ascontiguousarray(c0.reshape(NK, P).T, np.float32)
        c0_head = c0_vec if head else np.zeros_like(c0_vec)

        o0 = 0 if c < 4 else 512
        consts = np.zeros((P, 664), np.float32)
        consts[:, 0:NGT] = b_all.reshape(NGT, P).T
        consts[:, 24:536] = bo[o0:o0 + 512][None, :]
        consts[:, 536:664] = 1.0
        maps.append({
            "xT16": np.ascontiguousarray(x_loc.T.astype(f16)),
            "w16": np.ascontiguousarray(w_all.T.astype(f16)),
            "u16": np.ascontiguousarray(u_all.T.astype(f16)),
            "u8": np.ascontiguousarray(
                u_all.T.astype(ml_dtypes.float8_e4m3)),
            "wouo16": np.ascontiguousarray(
                wouoT_full[:, :, o0:o0 + 512].astype(f16)),
            "consts": consts,
            "sel": sel,
            "c0_vec": c0_vec,
            "c0_head": c0_head,
        })
    return maps


_CACHED = {}


def kernel(**inputs) -> np.ndarray:
    _CACHED["inputs"] = inputs
    if os.environ.get("KERNEL_SPMD_PATH", "0") == "1":
        in_maps = _prep_inputs(inputs)
        nc = build_kernel(SCHEDULE)
        legalize_waits(nc)
        res = run_bass_kernel_spmd(nc, in_maps, core_ids=list(range(NCORES)),
                                   trace=False)
        results = res.results
    else:
        results, times = timed_run(n_iters=250)
        _CACHED["times"] = times
    out = np.concatenate(
        [np.concatenate([results[c]["out_part"], results[c + 4]["out_part"]],
                        axis=1) for c in range(4)], axis=0)
    return out.astype(np.float32)


if __name__ == "__main__":
    nc = build_kernel(SCHEDULE)
    print("built ok; instructions:",
          sum(len(b.instructions) for b in nc.main_func.blocks))


def timed_run(n_iters=3):
    """Compile once, keep inputs device-resident, time executions.

    Returns (results_for_cores, [per-iter seconds])."""
    import time
    import jax
    from jax.sharding import Mesh, PartitionSpec, NamedSharding
    from jax.experimental.shard_map import shard_map
    from concourse import bass2jax

    inputs = _CACHED["inputs"]
    in_maps = _prep_inputs(inputs)
    nc = build_kernel(SCHEDULE)
    legalize_waits(nc)
    bass2jax.install_neuronx_cc_hook()

    partition_name = nc.partition_id_tensor.name if nc.partition_id_tensor else None
    in_names, out_names, out_avals, zero_outs = [], [], [], []
    import concourse.mybir as mybir_
    for alloc in nc.m.functions[0].allocations:
        if not isinstance(alloc, mybir_.MemoryLocationSet):
            continue
        name = alloc.memorylocations[0].name
        if alloc.kind == "ExternalInput":
            if name != partition_name:
                in_names.append(name)
        elif alloc.kind == "ExternalOutput":
            shape = tuple(alloc.tensor_shape)
            dtype = mybir_.dt.np(alloc.dtype)
            out_names.append(name)
            out_avals.append(jax.core.ShapedArray(shape, dtype))
            zero_outs.append(np.zeros(shape, dtype))
    n_params = len(in_names)
    all_in_names = list(in_names) + list(out_names)
    if partition_name is not None:
        all_in_names.append(partition_name)

    def _body(*args):
        operands = list(args)
        if partition_name is not None:
            operands.append(bass2jax.partition_id_tensor())
        outs = bass2jax._bass_exec_p.bind(
            *operands,
            out_avals=tuple(out_avals),
            in_names=tuple(all_in_names),
            out_names=tuple(out_names),
            lowering_input_output_aliases=(),
            sim_require_finite=True,
            sim_require_nnan=True,
            nc=nc,
        )
        return tuple(outs)

    devices = jax.devices()[:NCORES]
    mesh = Mesh(np.asarray(devices), ("core",))
    in_specs = (PartitionSpec("core"),) * (n_params + len(out_names))
    out_specs = (PartitionSpec("core"),) * len(out_names)
    fn = jax.jit(
        shard_map(_body, mesh=mesh, in_specs=in_specs, out_specs=out_specs,
                  check_rep=False),
        keep_unused=True,
    )
    concat_in = [
        np.concatenate([np.asarray(in_maps[c][nm])[None] for c in range(NCORES)],
                       axis=0).reshape(-1, *np.asarray(in_maps[0][nm]).shape[1:])
        for nm in in_names
    ]
    sh = NamedSharding(mesh, PartitionSpec("core"))
    dev_in = [jax.device_put(a, sh) for a in concat_in]
    dev_zero = [jax.device_put(
        np.zeros((NCORES * z.shape[0],) + z.shape[1:], z.dtype), sh)
        for z in zero_outs]
    times = []
    out_arrs = None
    for i in range(n_iters):
        t0 = time.time()
        out_arrs = fn(*dev_in, *dev_zero)
        jax.block_until_ready(out_arrs)
        times.append(time.time() - t0)
    results = [
        {nm: np.asarray(out_arrs[i]).reshape(NCORES, *out_avals[i].shape)[c]
         for i, nm in enumerate(out_names)}
        for c in range(NCORES)
    ]
    return results, times
